# revision 14
# baseline (speedup 1.0000x reference)
"""Self-contained Trainium2 Bass kernel for a 3-layer DGL-style GCN + NLL loss.

Strategy (8 NeuronCores, SPMD, dst-sharded graph):
  - Nodes re-labeled into a [chunk][core][window][128] slot layout: 98 windows
    of 128 slots per core (12544 slots/core, 12500 real).  5 chunks double as
    (a) AllGather chunking between layers and (b) the 5 gather sub-tables
    (each < 32768 rows so indices fit int16).  5 segments (vs 4) drop the
    per-(window, seg, band) edge count to ~104, so nearly every run fits ONE
    128-edge tile -> ~55% fewer padded tiles than a 4-way split.
  - Edges (dst-sorted) partitioned per core by (window, src-segment,
    dst-band) where band = lo/hi 64 nodes of the window.  Each (w,s,b) run is
    padded to 128-edge tiles (idx 0, weight 0).
  - Gathers batched: per (group of 14 windows, segment), tiles are gathered
    in 1024-index calls (the hard dma_gather cap: 64 descriptors/engine =
    the SDMA single-packet ceiling) -> 245 calls/layer vs the baseline's
    ~400 partial calls (~2.3 us of serialized GPSIMD engine time per call
    is the dominant cost; gather pads are SPREAD across table rows since
    identical pad indices serialize on one HBM bank).
  - Weighted one-hot S_w built banded: per window two DVE ops over
    [128, Tw, 64] (64-wide bands) instead of 128-wide.
  - SpMM accumulation per tile: aggT[D, band] += g[e,D].T @ S_w[e, band].
    First tile of each window runs full-width (its band + a zeros slot via a
    strided rhs AP) with start=True so the whole PSUM tile is initialized.
  - Dense layer: h = relu(aggT.T @ W + b); layer 3 stores logits to SBUF and
    runs one batched masked-NLL tail at the end; each core emits a partial
    NLL sum, host sums / N.
"""

import numpy as np

N = 100000
E = 1600000
D = 128
C = 40
NCORES = 8
RPC = 12500            # real nodes per core
WPC = 98               # windows per core
PW = 128               # nodes per window
NPC = WPC * PW         # 12544 slots per core
NP = NCORES * NPC      # 100352 total slots
CH_W = [20, 20, 20, 20, 18]       # windows per chunk (= gather segment)
NSEG = len(CH_W)
CH_W0 = np.concatenate([[0], np.cumsum(CH_W)[:-1]]).astype(np.int64)
CH_ROWS = [w * PW * NCORES for w in CH_W]      # rows per chunk region
CH_BASE = np.concatenate([[0], np.cumsum(CH_ROWS)]).astype(np.int64)
GW = 14                # windows per gather group
NGRP = WPC // GW       # 14 groups

LAST_EXEC_NS = None
LAST_RESULT = None


def _chunk_of_window(w):
    for c in range(NSEG):
        if CH_W0[c] <= w < CH_W0[c] + CH_W[c]:
            return c
    raise AssertionError(w)


CHUNK_OF_W = np.array([_chunk_of_window(w) for w in range(WPC)])


def _slot_rows(node):
    """Global table row for each original node id (vectorized)."""
    node = np.asarray(node, dtype=np.int64)
    k = node // RPC
    off = node % RPC
    w = off // PW
    p = off % PW
    c = CHUNK_OF_W[w]
    return CH_BASE[c] + k * (np.array(CH_W)[c] * PW) + (w - CH_W0[c]) * PW + p


def kernel(features, edge_w, W1, b1, W2, b2, W3, b3, src, dst, labels):
    import sys
    for p in ("/opt/trn_rl_repo",):
        if p not in sys.path:
            sys.path.insert(0, p)
    import ml_dtypes
    import concourse.bass as bass
    import concourse.bacc as bacc
    import concourse.mybir as mybir
    import concourse.tile as tile
    from concourse.bass_utils import run_bass_kernel_spmd

    bf16 = mybir.dt.bfloat16
    f32 = mybir.dt.float32
    i16 = mybir.dt.int16

    features = np.asarray(features, dtype=np.float32)
    edge_w = np.asarray(edge_w, dtype=np.float32)
    W1 = np.asarray(W1, dtype=np.float32); b1 = np.asarray(b1, dtype=np.float32)
    W2 = np.asarray(W2, dtype=np.float32); b2 = np.asarray(b2, dtype=np.float32)
    W3 = np.asarray(W3, dtype=np.float32); b3 = np.asarray(b3, dtype=np.float32)
    src = np.asarray(src, dtype=np.int64)
    dst = np.asarray(dst, dtype=np.int64)
    labels = np.asarray(labels, dtype=np.int64)

    # ---------------- host-side graph preprocessing ----------------
    src_row = _slot_rows(src)
    src_seg = np.searchsorted(CH_BASE[1:], src_row, side="right")
    src_idx = (src_row - CH_BASE[src_seg]).astype(np.int64)

    dst_off = dst % RPC
    dst_win = dst_off // PW
    dst_loc = dst_off % PW
    dst_band = (dst_loc >= 64).astype(np.int64)

    # per-edge run id: (window, segment, band)
    NRUN = WPC * NSEG * 2
    run = dst_win * (NSEG * 2) + src_seg * 2 + dst_band

    core_bounds = np.searchsorted(dst, np.arange(NCORES + 1) * RPC)
    cnt = np.zeros((NCORES, NRUN), dtype=np.int64)
    order_per_core = []
    for k in range(NCORES):
        s0, s1 = core_bounds[k], core_bounds[k + 1]
        r = run[s0:s1]
        o = np.argsort(r, kind="stable") + s0
        order_per_core.append(o)
        cnt[k] = np.bincount(r, minlength=NRUN)

    cnt_max = cnt.max(axis=0)                      # [NRUN]
    Trun = -(-cnt_max // PW)                       # tiles per run
    Trun = Trun.reshape(WPC, NSEG, 2)
    # force the window's first run (s=0, lo) to have >= 1 tile so tile 0
    # always exists for the start=True full-width matmul
    Trun[:, 0, 0] = np.maximum(Trun[:, 0, 0], 1)
    Trun_f = Trun.reshape(WPC, NSEG * 2)

    Tw = Trun_f.sum(axis=1)                        # window tile counts
    TWMAX = int(Tw.max())
    ot = np.concatenate([[0], np.cumsum(Tw)]).astype(np.int64)   # window-major tile offset
    TC = int(ot[-1])

    # gather (group-major) slot order: group -> seg -> window -> band -> tiles
    gslot_of = np.zeros((WPC, TWMAX), dtype=np.int64)   # group-LOCAL slot
    grp_seg_off = np.zeros((NGRP, NSEG + 1), dtype=np.int64)
    grp_tiles = np.zeros(NGRP, dtype=np.int64)
    gorder_runs = []     # per group: [(w, s, b, group_slot_base)]
    for gi in range(NGRP):
        ws = range(gi * GW, (gi + 1) * GW)
        pos = 0
        runs = []
        for s in range(NSEG):
            grp_seg_off[gi, s] = pos
            for w in ws:
                for b in range(2):
                    T = int(Trun[w, s, b])
                    if T == 0:
                        continue
                    lt = int(Trun_f[w, : s * 2 + b].sum())
                    for t in range(T):
                        gslot_of[w, lt + t] = pos + t
                    runs.append((w, s, b, pos))
                    pos += T
        grp_seg_off[gi, NSEG] = pos
        grp_tiles[gi] = pos
        gorder_runs.append(runs)
    grp_base = np.concatenate([[0], np.cumsum(grp_tiles)]).astype(np.int64)
    TGMAX = int(grp_tiles.max())
    assert int(grp_base[-1]) == TC

    # host tables
    IC = TC * 8                                    # int16 cols (128 idx -> 8 cols)
    IDX = np.zeros((NCORES, 128, IC), dtype=np.int16)
    DSTL = np.zeros((NCORES, 128, TC), dtype=ml_dtypes.bfloat16)
    WGT = np.zeros((NCORES, 128, TC), dtype=ml_dtypes.bfloat16)

    for k in range(NCORES):
        o = order_per_core[k]
        e_idx = src_idx[o]
        e_dl = dst_loc[o]
        e_w = edge_w[o]
        run_start = np.concatenate([[0], np.cumsum(cnt[k])]).astype(np.int64)
        for gi in range(NGRP):
            for (w, s, b, slot0) in gorder_runs[gi]:
                rid = w * (NSEG * 2) + s * 2 + b
                n = int(cnt[k, rid])
                T = int(Trun[w, s, b])
                cap = T * PW
                p0 = int(run_start[rid])
                # gather indices; pads point at SPREAD dummy rows (weight 0
                # kills their contribution) -- identical pad indices would
                # hammer one HBM bank and serialize the whole gather stream
                lst = (np.arange(cap, dtype=np.int64) * 997
                       % CH_ROWS[s]).astype(np.int16)
                lst[:n] = e_idx[p0:p0 + n].astype(np.int16)
                wrapped = lst.reshape(cap // 16, 16).T          # [16, T*8]
                cb = int((grp_base[gi] + slot0) * 8)
                IDX[k, :, cb:cb + cap // 16] = np.tile(wrapped, (8, 1))
                # window-major S_w metadata (band-local dst, weight)
                lt = int(Trun_f[w, : s * 2 + b].sum())
                col0 = int(ot[w]) + lt
                j = np.arange(n)
                DSTL[k, j % PW, col0 + j // PW] = (e_dl[p0:p0 + n] - 64 * b).astype(
                    np.float32)
                WGT[k, j % PW, col0 + j // PW] = e_w[p0:p0 + n].astype(np.float32)

    # features table in slot layout
    FEAT = np.zeros((NP, D), dtype=ml_dtypes.bfloat16)
    rows_all = _slot_rows(np.arange(N))
    FEAT[rows_all] = features.astype(ml_dtypes.bfloat16)

    # labels / mask per (core, partition, window)
    LBL = np.zeros((NCORES, 128, WPC), dtype=np.float32)
    MASK = np.zeros((NCORES, 128, WPC), dtype=np.float32)
    nn = np.arange(N)
    kk = nn // RPC
    off = nn % RPC
    LBL[kk, off % PW, off // PW] = labels.astype(np.float32)
    MASK[kk, off % PW, off // PW] = 1.0

    W1b = W1.astype(ml_dtypes.bfloat16)
    W2b = W2.astype(ml_dtypes.bfloat16)
    W3b = W3.astype(ml_dtypes.bfloat16)
    B1b = b1.reshape(1, -1).astype(ml_dtypes.bfloat16)
    B2b = b2.reshape(1, -1).astype(ml_dtypes.bfloat16)
    B3b = b3.reshape(1, -1).astype(ml_dtypes.bfloat16)

    # ---------------- bass program ----------------
    nc = bacc.Bacc("TRN2", target_bir_lowering=False, debug=False,
                   num_devices=NCORES, num_swdge_queues=4)

    feat_t = nc.dram_tensor("feat", [NP, D], bf16, kind="ExternalInput")
    idx_t = nc.dram_tensor("idx", [128, IC], i16, kind="ExternalInput")
    dstl_t = nc.dram_tensor("dstl", [128, TC], bf16, kind="ExternalInput")
    wgt_t = nc.dram_tensor("wgt", [128, TC], bf16, kind="ExternalInput")
    lbl_t = nc.dram_tensor("lbl", [128, WPC], f32, kind="ExternalInput")
    mask_t = nc.dram_tensor("mask", [128, WPC], f32, kind="ExternalInput")
    w1_t = nc.dram_tensor("w1", [D, D], bf16, kind="ExternalInput")
    w2_t = nc.dram_tensor("w2", [D, D], bf16, kind="ExternalInput")
    w3_t = nc.dram_tensor("w3", [D, C], bf16, kind="ExternalInput")
    b1_t = nc.dram_tensor("bb1", [1, D], bf16, kind="ExternalInput")
    b2_t = nc.dram_tensor("bb2", [1, D], bf16, kind="ExternalInput")
    b3_t = nc.dram_tensor("bb3", [1, C], bf16, kind="ExternalInput")
    out_t = nc.dram_tensor("out", [1, 1], f32, kind="ExternalOutput")

    def bcast_ap(ap, inner):
        """append a step-0 inner dim of size `inner`"""
        return bass.AP(ap.tensor, ap.offset, list(ap.ap) + [[0, inner]])

    def rep_ap(ap, times):
        """insert a step-0 middle dim (repeat a [128, X] tile) -> [128, times, X]"""
        return bass.AP(ap.tensor, ap.offset, [ap.ap[0], [0, times], ap.ap[1]])

    with tile.TileContext(nc) as tc:
        with (
            tc.tile_pool(name="const", bufs=1) as cpool,
            tc.tile_pool(name="gb", bufs=2) as gpool,
            tc.tile_pool(name="ix", bufs=2) as ixpool,
            tc.tile_pool(name="s01", bufs=2) as s01pool,
            tc.tile_pool(name="sw", bufs=2) as swpool,
            tc.tile_pool(name="small", bufs=2) as spool,
            tc.tile_pool(name="nll", bufs=1) as npool,
            tc.tile_pool(name="ps_agg", bufs=2, space="PSUM") as ps_agg,
            tc.tile_pool(name="ps_h", bufs=2, space="PSUM") as ps_h,
            tc.tile_pool(name="dram", bufs=1, space="DRAM") as dram,
        ):
            # ---- resident metadata ----
            dstl_s = cpool.tile([128, TC], bf16)
            wgt_s = cpool.tile([128, TC], bf16)
            lbl_s = cpool.tile([128, WPC], f32)
            mask_s = cpool.tile([128, WPC], f32)
            nc.sync.dma_start(out=dstl_s[:], in_=dstl_t[:])
            nc.sync.dma_start(out=wgt_s[:], in_=wgt_t[:])
            nc.sync.dma_start(out=lbl_s[:], in_=lbl_t[:])
            nc.sync.dma_start(out=mask_s[:], in_=mask_t[:])
            w_s = [cpool.tile([D, D], bf16, tag="w1", name="w1s"),
                   cpool.tile([D, D], bf16, tag="w2", name="w2s"),
                   cpool.tile([D, C], bf16, tag="w3", name="w3s")]
            nc.sync.dma_start(out=w_s[0][:], in_=w1_t[:])
            nc.sync.dma_start(out=w_s[1][:], in_=w2_t[:])
            nc.sync.dma_start(out=w_s[2][:], in_=w3_t[:])
            b_s = [cpool.tile([1, D], bf16, tag="b1", name="b1s"),
                   cpool.tile([1, D], bf16, tag="b2", name="b2s"),
                   cpool.tile([1, C], bf16, tag="b3", name="b3s")]
            nc.sync.dma_start(out=b_s[0][:], in_=b1_t[:])
            nc.sync.dma_start(out=b_s[1][:], in_=b2_t[:])
            nc.sync.dma_start(out=b_s[2][:], in_=b3_t[:])

            # tiled 64-iota [128, TWMAX, 64]: value = inner index
            iota_full = cpool.tile([128, TWMAX, 64], bf16)
            nc.gpsimd.iota(iota_full[:], pattern=[[0, TWMAX], [1, 64]], base=0,
                           channel_multiplier=0,
                           allow_small_or_imprecise_dtypes=True)
            iota40 = cpool.tile([128, C], f32)
            nc.gpsimd.iota(iota40[:], pattern=[[1, C]], base=0,
                           channel_multiplier=0,
                           allow_small_or_imprecise_dtypes=True)
            ones1 = cpool.tile([1, 128], bf16)
            nc.vector.memset(ones1[:], 1.0)
            onescol = cpool.tile([128, 1], f32)
            nc.vector.memset(onescol[:], 1.0)

            logits_all = cpool.tile([128, WPC, C], bf16)
            scratch = cpool.tile([128, WPC, C], bf16)

            # zero-fill gather + swt buffers once (swt zero slot must persist)
            for zi in range(2):
                t = gpool.tile([128, TGMAX, 128], bf16, tag="g", name=f"gz{zi}")
                nc.vector.memset(t[:], 0.0)
                t2 = swpool.tile([128, TWMAX + 1, 64], bf16, tag="swt",
                                 name=f"swz{zi}")
                nc.vector.memset(t2[:], 0.0)

            # ---- inter-layer DRAM tables ----
            h_mine = [[dram.tile([CH_W[c] * PW, D], bf16, tag=f"hm{l}{c}",
                                 name=f"hm{l}{c}")
                       for c in range(NSEG)] for l in range(2)]
            h_full = [[dram.tile([CH_ROWS[c], D], bf16, tag=f"hf{l}{c}",
                                 name=f"hf{l}{c}", addr_space="Shared")
                       for c in range(NSEG)] for l in range(2)]

            qcounter = [0]
            rg = [list(range(NCORES))]

            ICG = TGMAX * 8

            def do_group(gi, table_aps, layer):
                g = gpool.tile([128, TGMAX, 128], bf16, tag="g")
                ib = int(grp_base[gi] * 8)
                ie = int(grp_base[gi + 1] * 8)
                idx_g = ixpool.tile([128, ICG], i16, tag="idx")
                nc.sync.dma_start(out=idx_g[:, :ie - ib], in_=idx_t[:, ib:ie])
                for s in range(NSEG):
                    o0 = int(grp_seg_off[gi, s])
                    o1 = int(grp_seg_off[gi, s + 1])
                    nst = o1 - o0
                    if nst == 0:
                        continue
                    # single_packet=True caps one call at 64 descs/engine
                    # (1024 idx, the SDMA packet ceiling) but amortizes the
                    # ~57-cycle per-packet cost; issue ceil(nst/8) calls
                    tt = o0
                    while tt < o1:
                        tn = min(8, o1 - tt)
                        nidx = tn * PW
                        cb = tt * 8
                        nc.gpsimd.dma_gather(
                            g[:, tt:tt + tn, :],
                            table_aps[s],
                            idx_g[:, cb:cb + nidx // 16],
                            nidx, nidx, D,
                            queue_num=qcounter[0] % 4,
                        )
                        qcounter[0] += 1
                        tt += tn
                for w in range(gi * GW, (gi + 1) * GW):
                    do_window(w, g, layer)

            def do_window(w, g, layer):
                Tww = int(Tw[w])
                t0 = int(ot[w])
                s01 = s01pool.tile([128, TWMAX, 64], bf16, tag="s01")
                swt = swpool.tile([128, TWMAX + 1, 64], bf16, tag="swt")
                nc.vector.tensor_tensor(
                    out=s01[:, :Tww, :],
                    in0=iota_full[:, :Tww, :],
                    in1=bcast_ap(dstl_s[:, t0:t0 + Tww], 64),
                    op=mybir.AluOpType.is_equal,
                )
                nc.vector.tensor_tensor(
                    out=swt[:, :Tww, :],
                    in0=s01[:, :Tww, :],
                    in1=bcast_ap(wgt_s[:, t0:t0 + Tww], 64),
                    op=mybir.AluOpType.mult,
                )
                # per-tile banded SpMM accumulation
                agg = ps_agg.tile([128, 128], f32)
                sw_ap = swt[:]
                rhs0 = bass.AP(sw_ap.tensor, sw_ap.offset,
                               [sw_ap.ap[0], [TWMAX * 64, 2], [1, 64]])
                lt = 0
                for s in range(NSEG):
                    for b in range(2):
                        T = int(Trun[w, s, b])
                        for t in range(T):
                            gsl = int(gslot_of[w, lt])
                            if lt == 0:
                                # full width = [lo band | zeros slot], clears PSUM
                                nc.tensor.matmul(
                                    out=agg[:],
                                    lhsT=g[:, gsl, :],
                                    rhs=rhs0,
                                    start=True,
                                    stop=(Tww == 1),
                                )
                            else:
                                nc.tensor.matmul(
                                    out=agg[:, 64 * b:64 * b + 64],
                                    lhsT=g[:, gsl, :],
                                    rhs=swt[:, lt, :],
                                    start=False,
                                    stop=(lt == Tww - 1),
                                )
                            lt += 1
                assert lt == Tww
                aggT_sb = spool.tile([128, 128], bf16, tag="aggT")
                nc.scalar.copy(aggT_sb[:], agg[:])
                Dout = C if layer == 2 else D
                ph = ps_h.tile([128, 128], f32)
                nc.tensor.matmul(out=ph[:, :Dout], lhsT=aggT_sb[:],
                                 rhs=w_s[layer][:], start=True, stop=False)
                nc.tensor.matmul(out=ph[:, :Dout], lhsT=ones1[:],
                                 rhs=b_s[layer][:], start=False, stop=True)
                if layer < 2:
                    ht = spool.tile([128, D], bf16, tag="ht")
                    nc.scalar.activation(ht[:], ph[:, :D],
                                         mybir.ActivationFunctionType.Relu)
                    c = int(CHUNK_OF_W[w])
                    r0 = (w - int(CH_W0[c])) * PW
                    nc.sync.dma_start(out=h_mine[layer][c][r0:r0 + PW, :],
                                      in_=ht[:])
                else:
                    nc.scalar.copy(logits_all[:, w, :], ph[:, :C])

            # ---------------- the three layers ----------------
            feat_tabs = [feat_t[int(CH_BASE[s]):int(CH_BASE[s + 1]), :]
                         for s in range(NSEG)]
            import os
            dbg = os.environ.get("GCN_DEBUG", "")
            n_layers = {"L1": 1, "L12": 2}.get(dbg, 3)
            for layer in range(n_layers):
                if layer == 0:
                    tabs = feat_tabs
                else:
                    tabs = [h_full[layer - 1][s][:] for s in range(NSEG)]
                done_w = 0
                for gi in range(NGRP):
                    do_group(gi, tabs, layer)
                    done_w += GW
                    # fire AllGathers as soon as a chunk's windows are done
                    if layer < 2:
                        for c in range(NSEG):
                            end_w = int(CH_W0[c]) + CH_W[c]
                            if done_w - GW < end_w <= done_w:
                                nc.gpsimd.collective_compute(
                                    "AllGather", mybir.AluOpType.bypass,
                                    replica_groups=rg,
                                    ins=[h_mine[layer][c].opt()],
                                    outs=[h_full[layer][c].opt()],
                                )

            # ---------------- batched masked-NLL tail ----------------
            if n_layers == 3:
                mx = npool.tile([128, WPC], f32, tag="mx")
                nc.vector.tensor_reduce(out=mx[:], in_=logits_all[:],
                                        axis=mybir.AxisListType.X,
                                        op=mybir.AluOpType.max)
                nc.vector.tensor_tensor(out=scratch[:], in0=logits_all[:],
                                        in1=bcast_ap(mx[:], C),
                                        op=mybir.AluOpType.subtract)
                nc.scalar.activation(scratch[:], scratch[:],
                                     mybir.ActivationFunctionType.Exp)
                sumexp = npool.tile([128, WPC], f32, tag="sumexp")
                nc.vector.tensor_reduce(out=sumexp[:], in_=scratch[:],
                                        axis=mybir.AxisListType.X,
                                        op=mybir.AluOpType.add)
                lse = npool.tile([128, WPC], f32, tag="lse")
                nc.scalar.activation(lse[:], sumexp[:],
                                     mybir.ActivationFunctionType.Ln)
                nc.vector.tensor_tensor(out=scratch[:], in0=rep_ap(iota40[:], WPC),
                                        in1=bcast_ap(lbl_s[:], C),
                                        op=mybir.AluOpType.is_equal)
                nc.vector.tensor_tensor(out=scratch[:], in0=scratch[:],
                                        in1=logits_all[:],
                                        op=mybir.AluOpType.mult)
                picked = npool.tile([128, WPC], f32, tag="picked")
                nc.vector.tensor_reduce(out=picked[:], in_=scratch[:],
                                        axis=mybir.AxisListType.X,
                                        op=mybir.AluOpType.add)
                t1 = npool.tile([128, WPC], f32, tag="t1")
                nc.vector.tensor_tensor(out=t1[:], in0=lse[:], in1=mx[:],
                                        op=mybir.AluOpType.add)
                nc.vector.tensor_tensor(out=t1[:], in0=t1[:], in1=picked[:],
                                        op=mybir.AluOpType.subtract)
                nc.vector.tensor_tensor(out=t1[:], in0=t1[:], in1=mask_s[:],
                                        op=mybir.AluOpType.mult)
                nll_acc = npool.tile([128, 1], f32, tag="nllacc")
                nc.vector.tensor_reduce(out=nll_acc[:], in_=t1[:],
                                        axis=mybir.AxisListType.X,
                                        op=mybir.AluOpType.add)
                pscalar = ps_h.tile([1, 1], f32, tag="pscalar")
                nc.tensor.matmul(out=pscalar[:], lhsT=nll_acc[:], rhs=onescol[:],
                                 start=True, stop=True)
                res_sb = spool.tile([1, 1], f32, tag="res")
                nc.scalar.copy(res_sb[:], pscalar[:])
                nc.sync.dma_start(out=out_t[:], in_=res_sb[:])
            else:
                res_sb = spool.tile([1, 1], f32, tag="res")
                nc.vector.memset(res_sb[:], 0.0)
                nc.sync.dma_start(out=out_t[:], in_=res_sb[:])

    nc.compile()

    in_maps = []
    for k in range(NCORES):
        in_maps.append({
            "feat": FEAT, "idx": IDX[k], "dstl": DSTL[k], "wgt": WGT[k],
            "lbl": LBL[k], "mask": MASK[k],
            "w1": W1b, "w2": W2b, "w3": W3b,
            "bb1": B1b, "bb2": B2b, "bb3": B3b,
        })
    trace_ok = False
    try:
        from antenv.axon_hooks import get_axon_ntff_profile_hook
        trace_ok = get_axon_ntff_profile_hook() is not None
    except Exception:
        pass
    res = run_bass_kernel_spmd(nc, in_maps, list(range(NCORES)), trace=trace_ok)
    global LAST_EXEC_NS, LAST_RESULT
    LAST_EXEC_NS = res.exec_time_ns
    LAST_RESULT = res
    total = sum(float(res.results[k]["out"][0, 0]) for k in range(NCORES))
    return np.float32(total / N)
